# revision 11
# baseline (speedup 1.0000x reference)
"""Trainium2 Bass kernel for nn_DeformConv_23278722744918.

The reference passes raw integer pixel coordinates to grid_sample as if they
were normalized [-1,1] coords (align_corners=True). After de-normalization,
xpix = (clip(h+i,0,95)+1)*47.5 and ypix = (clip(w+j,0,95)+1)*47.5, so every
sample with h+i >= 2 or w+j >= 2 lands outside [0,95] and is zero
(padding_mode='zeros').  Only four tap values survive, shared by all (h,w):

  A = 0.25*(x[47,47]+x[47,48]+x[48,47]+x[48,48])   (coord cases 0,0)
  B = 0.50*(x[47,95]+x[48,95])                     (coord cases 1,0)
  C = 0.50*(x[95,47]+x[95,48])                     (coord cases 0,1)
  D =       x[95,95]                               (coord cases 1,1)

After the stride-3 VALID conv over the rearranged feature map, the output is
b_conv everywhere except the 2x2 corner (per batch, out-channel):

  out[b,o,0,0] = sum_c A*w00 + C*w01 + B*w10 + D*w11   (+ b_conv[o])
  out[b,o,0,1] = sum_c C*w00 + D*w10
  out[b,o,1,0] = sum_c B*w00 + D*w01
  out[b,o,1,1] = sum_c D*w00

(w_ij = w_conv[o,c,i,j]; the offset-conv branch is dead: + 0.0*sum(off).)

Sharding: output channels are split 8 ways across the NeuronCores (the batch
dim is only 4); the sampled rows of x (y=47,48,95) are replicated.  Each core
gathers its bilinear taps on-device (reduces + scales), runs the 4 corner
matmuls on the TensorEngine, fills its bias background and writes its
[4,8,96,96] output shard with disjoint DMAs.
"""

import numpy as np

B, IC, IH, IW = 4, 64, 96, 96
OC = 64
NCORES = 8
OCP = OC // NCORES  # out channels per core
HW = IH * IW        # 9216
QCH = HW // 4       # 2304: background tile free size (4 partition-chunks/plane)

_ROWS = (47, 48, 95)  # sampled rows of x (y coords); cols sampled: 47,48,95

_prog_cache = {}


def _build_program():
    """One SPMD Bass program: identical on every core; per-core data differs."""
    import concourse.bass as bass
    import concourse.bacc as bacc
    import concourse.mybir as mybir
    import concourse.tile as tile

    nc = bacc.Bacc()
    dt = mybir.dt.float32

    xr_d = nc.declare_dram_parameter("xr", [IC, B, 3, IW], dt, isOutput=False)
    wb_d = nc.declare_dram_parameter("wb", [IC, 32], dt, isOutput=False)
    b128_d = nc.declare_dram_parameter("bias128", [128, 1], dt, isOutput=False)
    bv_d = nc.declare_dram_parameter("biasV", [B, OCP, 2, 2], dt, isOutput=False)
    out_d = nc.declare_dram_parameter("out", [B, OCP, IH, IW], dt, isOutput=True)

    with tile.TileContext(nc) as tc:
        with (
            tc.tile_pool(name="sbuf", bufs=1) as pool,
            tc.tile_pool(name="psum", bufs=1, space=bass.MemorySpace.PSUM) as psum,
        ):
            xr = pool.tile([IC, B, 3, IW], dt)
            wb0 = pool.tile([IC, 32], dt)
            wb = pool.tile([IC, 32], dt)
            b128 = pool.tile([128, 1], dt)
            bv = pool.tile([B, OCP, 2, 2], dt)
            S = pool.tile([IC, 16], dt)
            V = pool.tile([B, OCP, 2, 2], dt)
            bg = pool.tile([128, QCH], dt)
            Vp = psum.tile([B, 32], dt)

            nc.sync.dma_start(xr[:], xr_d[:])
            nc.sync.dma_start(wb0[:], wb_d[:])
            nc.sync.dma_start(b128[:], b128_d[:])
            nc.sync.dma_start(bv[:], bv_d[:])

            # Bilinear taps.  S[c, tap*4+b], taps A,B,C,D; xr rows: 0->y47,
            # 1->y48, 2->y95; cols 47,48,95.
            # Funnel the weight DMA through the vector engine so every
            # matmul waits on a single semaphore (PE load-weights has one
            # sync-wait slot; S and wb then share the DVE sem).
            nc.vector.tensor_copy(wb[:], wb0[:])

            AX = mybir.AxisListType
            nc.vector.reduce_sum(S[:, 0:4], xr[:, :, 0:2, 47:49], axis=AX.XY)
            nc.vector.reduce_sum(S[:, 4:8], xr[:, :, 0:2, 95:96], axis=AX.XY)
            nc.vector.reduce_sum(S[:, 8:12], xr[:, :, 2:3, 47:49], axis=AX.XY)
            nc.vector.tensor_copy(S[:, 12:16], xr[:, :, 2:3, 95:96])
            nc.vector.tensor_scalar_mul(S[:, 0:4], S[:, 0:4], 0.25)
            nc.vector.tensor_scalar_mul(S[:, 4:12], S[:, 4:12], 0.5)

            # Corner values Vp[b, (h*2+w)*8+o], accumulated on the PE so all
            # engine operands start at partition 0 (BIR partition-alignment
            # rule forbids reading PSUM/SBUF at partition offset 8 etc.).
            # taps: A=0,B=1,C=2,D=3; w blocks: w00=0,w01=1,w10=2,w11=3.
            sc = lambda t: S[:, t * 4 : (t + 1) * 4]
            wc = lambda ij: wb[:, ij * 8 : (ij + 1) * 8]
            MM = nc.tensor.matmul
            MM(Vp[:, 0:8], sc(0), wc(0), start=True, stop=False)   # A*w00
            MM(Vp[:, 0:8], sc(2), wc(1), start=False, stop=False)  # C*w01
            MM(Vp[:, 0:8], sc(1), wc(2), start=False, stop=False)  # B*w10
            MM(Vp[:, 0:8], sc(3), wc(3), start=False, stop=True)   # D*w11
            MM(Vp[:, 8:16], sc(2), wc(0), start=True, stop=False)  # C*w00
            MM(Vp[:, 8:16], sc(3), wc(2), start=False, stop=True)  # D*w10
            MM(Vp[:, 16:24], sc(1), wc(0), start=True, stop=False)  # B*w00
            MM(Vp[:, 16:24], sc(3), wc(1), start=False, stop=True)  # D*w01
            MM(Vp[:, 24:32], sc(3), wc(0), start=True, stop=True)   # D*w00

            # Permute Vp's (h*2+w)*8+o free layout to V's (o,h,w) so the
            # corner DMA's last dim (w) is contiguous in SBUF.
            nc.vector.tensor_copy(
                V[:].rearrange("b o h w -> b o (h w)"),
                Vp[:].rearrange("b (hw o) -> b o hw", o=OCP),
            )
            nc.vector.tensor_add(V[:], V[:], bv[:])

            # Bias background: bg[q*32 + b*8 + o, r] = b_conv[o]
            nc.vector.memset(bg[:], 0.0)
            nc.scalar.activation(
                bg[:], bg[:], mybir.ActivationFunctionType.Identity, bias=b128[:]
            )

            # Output shard writes, all disjoint: plane hw = q*2304 + r,
            # corner positions hw in {0,1,96,97} come from V.
            ov = out_d[:].rearrange("b o h w -> (b o) (h w)")
            nc.sync.dma_start(ov[:, 2:96], bg[0:32, 2:96])
            nc.sync.dma_start(ov[:, 98:QCH], bg[0:32, 98:QCH])
            for q in (1, 2, 3):
                nc.sync.dma_start(
                    ov[:, q * QCH : (q + 1) * QCH], bg[q * 32 : (q + 1) * 32, :]
                )
            for h in (0, 1):
                nc.sync.dma_start(
                    out_d[:, :, h : h + 1, 0:2], V[:, :, h : h + 1, :]
                )

    nc.finalize()  # Bacc.finalize runs the wait-splitting legalization passes
    return nc


def _get_program():
    if "nc" not in _prog_cache:
        _prog_cache["nc"] = _build_program()
    return _prog_cache["nc"]


def _make_in_maps(x, w_conv, b_conv):
    x = np.ascontiguousarray(x, dtype=np.float32)
    w_conv = np.ascontiguousarray(w_conv, dtype=np.float32)
    b_conv = np.ascontiguousarray(b_conv, dtype=np.float32)

    xr = np.ascontiguousarray(x[:, :, _ROWS, :].transpose(1, 0, 2, 3))
    in_maps = []
    for core in range(NCORES):
        o0 = core * OCP
        wsl = w_conv[o0 : o0 + OCP, :, 0:2, 0:2]  # [8,64,2,2] (o,c,i,j)
        wb = np.ascontiguousarray(wsl.transpose(1, 2, 3, 0).reshape(IC, 32))
        bc8 = b_conv[o0 : o0 + OCP]
        bias128 = np.ascontiguousarray(np.tile(bc8, 16)[:, None])
        # biasV[b, o, h, w] = bc8[o]
        biasv = np.ascontiguousarray(
            np.broadcast_to(np.repeat(bc8, 4)[None, :], (B, 32))
        ).reshape(B, OCP, 2, 2)
        in_maps.append(
            {"xr": xr, "wb": wb, "bias128": bias128, "biasV": biasv}
        )
    return in_maps


def _run(x, w_conv, b_conv, trace=False, **spmd_kwargs):
    from concourse.bass_utils import run_bass_kernel_spmd

    nc = _get_program()
    in_maps = _make_in_maps(x, w_conv, b_conv)
    res = run_bass_kernel_spmd(
        nc, in_maps, core_ids=list(range(NCORES)), trace=trace, **spmd_kwargs
    )
    out = np.concatenate([r["out"] for r in res.results], axis=1)
    return out, res


def kernel(x, w_off, b_off, w_conv, b_conv):
    out, _ = _run(x, w_conv, b_conv, trace=False)
    return out


# revision 12
# speedup vs baseline: 1.4187x; 1.4187x over previous
"""Trainium2 Bass kernel for nn_DeformConv_23278722744918.

The reference passes raw integer pixel coordinates to grid_sample as if they
were normalized [-1,1] coords (align_corners=True). After de-normalization,
xpix = (clip(h+i,0,95)+1)*47.5 and ypix = (clip(w+j,0,95)+1)*47.5, so every
sample with h+i >= 2 or w+j >= 2 lands outside [0,95] and is zero
(padding_mode='zeros').  Only four tap values survive, shared by all (h,w):

  A = 0.25*(x[47,47]+x[47,48]+x[48,47]+x[48,48])   (coord cases 0,0)
  B = 0.50*(x[47,95]+x[48,95])                     (coord cases 1,0)
  C = 0.50*(x[95,47]+x[95,48])                     (coord cases 0,1)
  D =       x[95,95]                               (coord cases 1,1)

After the stride-3 VALID conv over the rearranged feature map, the output is
b_conv everywhere except the 2x2 corner (per batch, out-channel):

  out[b,o,0,0] = sum_c A*w00 + C*w01 + B*w10 + D*w11   (+ b_conv[o])
  out[b,o,0,1] = sum_c C*w00 + D*w10
  out[b,o,1,0] = sum_c B*w00 + D*w01
  out[b,o,1,1] = sum_c D*w00

(w_ij = w_conv[o,c,i,j]; the offset-conv branch is dead: + 0.0*sum(off).)

Sharding: output channels are split 8 ways across the NeuronCores (the batch
dim is only 4); the sampled rows of x (y=47,48,95) are replicated.  Each core
gathers its bilinear taps on-device (reduces + scales), runs the 4 corner
matmuls on the TensorEngine, fills its background tile and writes its
[4,8,96,96] output shard with disjoint DMAs spread over both HWDGE rings.

Two program variants: b_conv==0 (always true for this problem's
setup_inputs) uses a pure-memset background; nonzero b_conv broadcasts the
bias with a DVE copy (step-0 AP) instead.
"""

import numpy as np

B, IC, IH, IW = 4, 64, 96, 96
OC = 64
NCORES = 8
OCP = OC // NCORES  # out channels per core
HW = IH * IW        # 9216
QCH = HW // 4       # 2304: background tile free size (4 partition-chunks/plane)

_ROWS = (47, 48, 95)  # sampled rows of x (y coords); cols sampled: 47,48,95

_prog_cache = {}


def _build_program(with_bias):
    """One SPMD Bass program: identical on every core; per-core data differs."""
    import concourse.bacc as bacc
    import concourse.bass as bass
    import concourse.mybir as mybir
    import concourse.tile as tile

    nc = bacc.Bacc()
    dt = mybir.dt.float32

    xr_d = nc.declare_dram_parameter("xr", [IC, B, 3, IW], dt, isOutput=False)
    wb_d = nc.declare_dram_parameter("wb", [IC, 32], dt, isOutput=False)
    if with_bias:
        b128_d = nc.declare_dram_parameter("bias128", [128, 1], dt, isOutput=False)
        bv_d = nc.declare_dram_parameter("biasV", [B, OCP, 2, 2], dt, isOutput=False)
    out_d = nc.declare_dram_parameter("out", [B, OCP, IH, IW], dt, isOutput=True)

    with tile.TileContext(nc) as tc:
        with (
            tc.tile_pool(name="sbuf", bufs=1) as pool,
            tc.tile_pool(name="psum", bufs=1, space=bass.MemorySpace.PSUM) as psum,
        ):
            xr = pool.tile([IC, B, 3, IW], dt)
            wb0 = pool.tile([IC, 32], dt)
            wb = pool.tile([IC, 32], dt)
            S = pool.tile([IC, 16], dt)
            V = pool.tile([B, OCP, 2, 2], dt)
            bg = pool.tile([128, QCH], dt)
            Vp = psum.tile([B, 32], dt)

            # Background: bg[q*32 + b*8 + o, r] = b_conv[o] (or just zeros).
            # The fill gates the big output writes, so keep it off the Scalar
            # engine (ACT table load + drain costs ~6us) and split the plain
            # memset across DVE and GpSimd.
            if with_bias:
                b128 = pool.tile([128, 1], dt)
                bv = pool.tile([B, OCP, 2, 2], dt)
                nc.sync.dma_start(b128[:], b128_d[:])
                nc.sync.dma_start(bv[:], bv_d[:])
                nc.vector.tensor_copy(bg[:], b128[:, 0:1].to_broadcast((128, QCH)))
            else:
                nc.vector.memset(bg[:, 0 : QCH // 2], 0.0)
                nc.gpsimd.memset(bg[:, QCH // 2 : QCH], 0.0)

            nc.sync.dma_start(xr[:], xr_d[:])
            nc.sync.dma_start(wb0[:], wb_d[:])

            # Funnel the weight DMA through the vector engine so every matmul
            # waits on a single semaphore (PE load-weights has one sync-wait
            # slot; S and wb then share the DVE sem).
            nc.vector.tensor_copy(wb[:], wb0[:])

            # Bilinear taps.  S[c, tap*4+b], taps A,B,C,D; xr rows: 0->y47,
            # 1->y48, 2->y95; cols 47,48,95.
            AX = mybir.AxisListType
            nc.vector.reduce_sum(S[:, 0:4], xr[:, :, 0:2, 47:49], axis=AX.XY)
            nc.vector.reduce_sum(S[:, 4:8], xr[:, :, 0:2, 95:96], axis=AX.XY)
            nc.vector.reduce_sum(S[:, 8:12], xr[:, :, 2:3, 47:49], axis=AX.XY)
            nc.vector.tensor_copy(S[:, 12:16], xr[:, :, 2:3, 95:96])
            nc.vector.tensor_scalar_mul(S[:, 0:4], S[:, 0:4], 0.25)
            nc.vector.tensor_scalar_mul(S[:, 4:12], S[:, 4:12], 0.5)

            # Corner values Vp[b, (h*2+w)*8+o], accumulated on the PE so all
            # engine operands start at partition 0 (BIR partition-alignment
            # rule forbids reading PSUM/SBUF at partition offset 8 etc.).
            # taps: A=0,B=1,C=2,D=3; w blocks: w00=0,w01=1,w10=2,w11=3.
            sc = lambda t: S[:, t * 4 : (t + 1) * 4]
            wc = lambda ij: wb[:, ij * 8 : (ij + 1) * 8]
            MM = nc.tensor.matmul
            MM(Vp[:, 0:8], sc(0), wc(0), start=True, stop=False)   # A*w00
            MM(Vp[:, 0:8], sc(2), wc(1), start=False, stop=False)  # C*w01
            MM(Vp[:, 0:8], sc(1), wc(2), start=False, stop=False)  # B*w10
            MM(Vp[:, 0:8], sc(3), wc(3), start=False, stop=True)   # D*w11
            MM(Vp[:, 8:16], sc(2), wc(0), start=True, stop=False)  # C*w00
            MM(Vp[:, 8:16], sc(3), wc(2), start=False, stop=True)  # D*w10
            MM(Vp[:, 16:24], sc(1), wc(0), start=True, stop=False)  # B*w00
            MM(Vp[:, 16:24], sc(3), wc(1), start=False, stop=True)  # D*w01
            MM(Vp[:, 24:32], sc(3), wc(0), start=True, stop=True)   # D*w00

            # Permute Vp's (h*2+w)*8+o free layout to V's (o,h,w) so the
            # corner DMA's last dim (w) is contiguous in SBUF.
            nc.vector.tensor_copy(
                V[:].rearrange("b o h w -> b o (h w)"),
                Vp[:].rearrange("b (hw o) -> b o hw", o=OCP),
            )
            if with_bias:
                nc.vector.tensor_add(V[:], V[:], bv[:])

            # Output shard writes, all disjoint: plane hw = q*2304 + r,
            # corner positions hw in {0,1,96,97} come from V.  Alternate the
            # two HWDGE rings (sync=SP, scalar=ACT) for parallel issue.
            ov = out_d[:].rearrange("b o h w -> (b o) (h w)")
            nc.sync.dma_start(ov[:, 2:96], bg[0:32, 2:96])
            nc.scalar.dma_start(ov[:, 98:QCH], bg[0:32, 98:QCH])
            nc.sync.dma_start(ov[:, QCH : 2 * QCH], bg[32:64, :])
            nc.scalar.dma_start(ov[:, 2 * QCH : 3 * QCH], bg[64:96, :])
            nc.sync.dma_start(ov[:, 3 * QCH : 4 * QCH], bg[96:128, :])
            for h in (0, 1):
                nc.scalar.dma_start(
                    out_d[:, :, h : h + 1, 0:2], V[:, :, h : h + 1, :]
                )

    nc.finalize()  # Bacc.finalize runs the wait-splitting legalization passes
    return nc


def _get_program(with_bias):
    key = bool(with_bias)
    if key not in _prog_cache:
        _prog_cache[key] = _build_program(key)
    return _prog_cache[key]


def _make_in_maps(x, w_conv, b_conv, with_bias=None):
    x = np.ascontiguousarray(x, dtype=np.float32)
    w_conv = np.ascontiguousarray(w_conv, dtype=np.float32)
    b_conv = np.ascontiguousarray(b_conv, dtype=np.float32)
    if with_bias is None:
        with_bias = bool(np.any(b_conv != 0))

    xr = np.ascontiguousarray(x[:, :, _ROWS, :].transpose(1, 0, 2, 3))
    in_maps = []
    for core in range(NCORES):
        o0 = core * OCP
        wsl = w_conv[o0 : o0 + OCP, :, 0:2, 0:2]  # [8,64,2,2] (o,c,i,j)
        wb = np.ascontiguousarray(wsl.transpose(1, 2, 3, 0).reshape(IC, 32))
        m = {"xr": xr, "wb": wb}
        if with_bias:
            bc8 = b_conv[o0 : o0 + OCP]
            m["bias128"] = np.ascontiguousarray(np.tile(bc8, 16)[:, None])
            # biasV[b, o, h, w] = bc8[o]
            m["biasV"] = np.ascontiguousarray(
                np.broadcast_to(np.repeat(bc8, 4)[None, :], (B, 32))
            ).reshape(B, OCP, 2, 2)
        in_maps.append(m)
    return in_maps


def _run(x, w_conv, b_conv, trace=False, **spmd_kwargs):
    from concourse.bass_utils import run_bass_kernel_spmd

    with_bias = bool(np.any(np.asarray(b_conv) != 0))
    nc = _get_program(with_bias)
    in_maps = _make_in_maps(x, w_conv, b_conv, with_bias)
    res = run_bass_kernel_spmd(
        nc, in_maps, core_ids=list(range(NCORES)), trace=trace, **spmd_kwargs
    )
    out = np.concatenate([r["out"] for r in res.results], axis=1)
    return out, res


def kernel(x, w_off, b_off, w_conv, b_conv):
    out, _ = _run(x, w_conv, b_conv, trace=False)
    return out
